# revision 17
# baseline (speedup 1.0000x reference)
"""Causal self-attention (B=4, S=2048, D=1024, H=16) on 8 NeuronCores.

Sharding: core c handles batch b = c//2 and head-group g = c%2 (8 heads).
Each core computes qkv for its head group, causal attention for its 8 heads,
and a partial projection (its 512 rows of W_proj). Host sums the two partial
outputs per batch and adds b_proj.

Device layout notes:
 - x is passed transposed (xT [D, S]) and bf16; qT/kT are computed in
   [qkv_col, token] layout so the scores matmul needs no transposes:
   scoresT[k_tok, q_tok] = kT_tile.T @ qT  (lhsT = kT, contraction = head dim).
 - softmax runs on scoresT: exp on ScalarE (scale=1/8 folded in), causal
   handled by a zero-prefix memset + one triangular 128x128 mask multiply on
   the diagonal block.
 - denominators come from a ones-column appended to v (v_aug [k,65]); the
   ctx matmul then yields [ctx(64 rows); sums(1 row)] per q block.
 - normalization: reciprocal of the sums row, partition-broadcast on GpSimd,
   one fused multiply+cast on VectorE.
"""

import numpy as np
import ml_dtypes

import concourse.bacc as bacc
import concourse.tile as tile
from concourse import mybir
from concourse.bass_utils import run_bass_kernel_spmd

BF16 = mybir.dt.bfloat16
F32 = mybir.dt.float32
EXP = mybir.ActivationFunctionType.Exp

B = 4
S = 2048  # tokens per batch
D = 1024
HG = 8    # heads per core
HD = 64
GC = HG * HD  # 512 qkv columns per core per q/k/v
N_CORES = 8
SCALE = 0.125  # 1/sqrt(64)


def _body(nc, xT, wq, wk, wv, wp, bqkv, tri, ident, outT, tc, layout="fill"):
    _const_cm = tc.tile_pool(name="const", bufs=1)
    const = _const_cm.__enter__()
    qT_sb = const.tile([128, 4, S], BF16)
    kT_sb = const.tile([128, 4, S], BF16)
    ctxT_sb = const.tile([128, 4, S], BF16)
    vaug_sb = const.tile([128, 16, HG, 65], BF16)
    wp_sb = const.tile([128, 4, D], BF16)
    tri_sb = const.tile([128, 128], BF16)
    ident_sb = const.tile([128, 128], BF16)
    b_sb = const.tile([1, 3 * GC], BF16)
    ones1 = const.tile([1, 512], BF16)

    nc.vector.memset(ones1[:], 1.0)
    nc.vector.memset(vaug_sb[:, :, :, 64:65], 1.0)
    nc.sync.dma_start(out=tri_sb[:], in_=tri.ap())
    nc.sync.dma_start(out=ident_sb[:], in_=ident.ap())
    nc.sync.dma_start(out=b_sb[:], in_=bqkv.ap())
    for ct in range(4):
        nc.sync.dma_start(out=wp_sb[:, ct, :], in_=wp.ap()[128 * ct:128 * (ct + 1), :])

    xT_sb = const.tile([128, 8, S], BF16)
    wq_sb = const.tile([128, 8, GC], BF16)
    wk_sb = const.tile([128, 8, GC], BF16)
    wv_sb = const.tile([128, 8, GC], BF16)
    for t in range(8):
        nc.sync.dma_start(out=xT_sb[:, t, :], in_=xT.ap()[128 * t:128 * (t + 1), :])
        nc.sync.dma_start(out=wq_sb[:, t, :], in_=wq.ap()[128 * t:128 * (t + 1), :])
        nc.sync.dma_start(out=wk_sb[:, t, :], in_=wk.ap()[128 * t:128 * (t + 1), :])
        nc.sync.dma_start(out=wv_sb[:, t, :], in_=wv.ap()[128 * t:128 * (t + 1), :])

    # One shared PSUM pool scheme across all phases so emission can pipeline:
    #   scp "sc": [128,1024] slots x2 (4 banks) - qkv psums, scores, proj
    #   cxp "cx": [65,512] slots x4 (4 banks)   - ctx accumulators
    _scp_cm = tc.tile_pool(name="scp", bufs=2, space="PSUM")
    scp = _scp_cm.__enter__()
    _cxp_cm = tc.tile_pool(name="cxp", bufs=4, space="PSUM")
    cxp = _cxp_cm.__enter__()
    _prp_cm = tc.tile_pool(name="prp", bufs=5)
    prp = _prp_cm.__enter__()
    _nrm_cm = tc.tile_pool(name="nrm", bufs=4)
    nrm = _nrm_cm.__enter__()

    def qk_group(c, qk, tb):
        w_sb, dst, boff = ((wq_sb, qT_sb, 0), (wk_sb, kT_sb, GC))[qk]
        ps = scp.tile([128, 512], F32, tag="sc", name=f"qk_{c}_{boff}_{tb}")
        for t in range(8):
            nc.tensor.matmul(
                ps[:],
                lhsT=w_sb[:, t, 128 * c:128 * (c + 1)],
                rhs=xT_sb[:, t, 512 * tb:512 * (tb + 1)],
                start=(t == 0), stop=False)
        nc.tensor.matmul(
            ps[:],
            lhsT=b_sb[0:1, boff + 128 * c: boff + 128 * (c + 1)],
            rhs=ones1[0:1, :],
            start=False, stop=True)
        nc.vector.tensor_copy(dst[:, c, 512 * tb:512 * (tb + 1)], ps[:])

    def v_tile(j):
        # v in natural [token, v_col] layout, + bias, scattered into v_aug
        psv = scp.tile([128, 512], F32, tag="sc", name=f"pv_{j}")
        for t in range(8):
            nc.tensor.matmul(
                psv[:],
                lhsT=xT_sb[:, t, 128 * j:128 * (j + 1)],
                rhs=wv_sb[:, t, :],
                start=(t == 0), stop=False)
        nc.tensor.matmul(
            psv[:],
            lhsT=ones1[0:1, 0:128],
            rhs=b_sb[0:1, 2 * GC:3 * GC],
            start=False, stop=True)
        nc.vector.tensor_copy(
            vaug_sb[:, j, :, 0:64],
            psv[:].rearrange("p (h c) -> p h c", h=HG))

    def normalize(h, qb, ctx_ps):
        o = 64 * (h % 2)
        c = h // 2
        rec = nrm.tile([1, 512], F32, tag="rec", name=f"rec_{h}_{qb}")
        nc.vector.reciprocal(rec[:], ctx_ps[qb][64:65, :])
        bc = nrm.tile([64, 512], F32, tag="bc", name=f"bc_{h}_{qb}")
        nc.gpsimd.partition_broadcast(bc[:], rec[:])
        if o == 0:
            nc.vector.tensor_mul(
                ctxT_sb[0:64, c, 512 * qb:512 * (qb + 1)],
                ctx_ps[qb][0:64, :], bc[:])
        else:
            stg = nrm.tile([64, 512], BF16, tag="stg", name=f"stg_{h}_{qb}")
            nc.vector.tensor_mul(stg[:], ctx_ps[qb][0:64, :], bc[:])
            nc.sync.dma_start(
                out=ctxT_sb[64:128, c, 512 * qb:512 * (qb + 1)], in_=stg[:])

    def head_block(h, filler=None):
        o = 64 * (h % 2)
        c = h // 2
        ctx_ps = [cxp.tile([65, 512], F32, tag="cx", name=f"cx_{h}_{qb}")
                  for qb in range(4)]
        for j in range(16):
            if filler is not None:
                filler(j)
            qbm, r = divmod(j, 4)
            width = S - 512 * qbm
            rel0 = 128 * r
            pT = prp.tile([128, S], BF16, tag="probs", name=f"pT_{h}_{j}")
            # scores chunks of <=1024 free, one exp per chunk; the causal mask
            # is applied in PSUM by adding tri_neg (0 / -1e9) to the diagonal
            # 128-wide band via an identity matmul, keeping the whole
            # scores->exp chain on PE->ACT only
            for ch0 in range(0, width, 1024):
                ch1 = min(ch0 + 1024, width)
                lo = max(ch0, rel0)
                if lo >= ch1:
                    continue
                ps = scp.tile([128, 1024], F32, tag="sc", name=f"sc_{h}_{j}_{ch0}")
                for qb in range(qbm + ch0 // 512, qbm + ch1 // 512):
                    rq0 = (qb - qbm) * 512
                    mlo = max(rq0, rel0)
                    diag = mlo == rel0 and ch0 == 0
                    nc.tensor.matmul(
                        ps[:, mlo - ch0: rq0 + 512 - ch0],
                        lhsT=kT_sb[o:o + 64, c, 128 * j:128 * (j + 1)],
                        rhs=qT_sb[o:o + 64, c,
                                  512 * qbm + mlo: 512 * qbm + rq0 + 512],
                        start=True, stop=not diag, skip_group_check=True)
                    if diag:
                        nc.tensor.matmul(
                            ps[:, rel0 - ch0: rel0 - ch0 + 128],
                            lhsT=ident_sb[:],
                            rhs=tri_sb[:],
                            start=False, stop=True, skip_group_check=True)
                nc.scalar.activation(
                    pT[:, lo:ch1], ps[:, lo - ch0:ch1 - ch0], EXP, scale=SCALE)
            # ctx accumulation (with sums in row 64); the diagonal block's
            # masked prefix [0, rel0) is never computed nor accumulated
            for qb in range(qbm, 4):
                lo = rel0 if qb == qbm else 0
                nc.tensor.matmul(
                    ctx_ps[qb][:, lo:512],
                    lhsT=vaug_sb[:, j, h, :],
                    rhs=pT[:, (qb - qbm) * 512 + lo: (qb - qbm + 1) * 512],
                    start=(j == 0), stop=(j == 4 * qb + 3))
            if r == 3:
                # qb = (j-3)//4 just received its last accumulation
                normalize(h, (j - 3) // 4, ctx_ps)

    def spread(groups):
        stride = max(1, 16 // max(1, len(groups)))
        def f(j):
            i = j // stride
            if j % stride == 0 and i < len(groups):
                groups[i]()
        return f

    qkg = [[(lambda c=c, qk=qk, tb=tb: qk_group(c, qk, tb))
            for qk in range(2) for tb in range(4)] for c in range(4)]
    if layout == "fill":
        # qk(0) upfront; v interleaved into h0; qk(1..3) spread into h1..h5
        for g in qkg[0]:
            g()
        head_block(0, filler=lambda j: v_tile(j))
        head_block(1, filler=spread(qkg[1]))
        head_block(2, filler=spread(qkg[2][:4]))
        head_block(3, filler=spread(qkg[2][4:]))
        head_block(4, filler=spread(qkg[3][:4]))
        head_block(5, filler=spread(qkg[3][4:]))
        head_block(6)
        head_block(7)
    elif layout == "seq":
        # all qkv upfront, then pure attention heads
        for c in range(4):
            for g in qkg[c]:
                g()
        for j in range(16):
            v_tile(j)
        for h in range(HG):
            head_block(h)
    elif layout == "block":
        # qkv blocks between head pairs
        for g in qkg[0]:
            g()
        for j in range(16):
            v_tile(j)
        for c in range(4):
            if c:
                for g in qkg[c]:
                    g()
            head_block(2 * c)
            head_block(2 * c + 1)
    else:
        raise ValueError(layout)

    # ---------------- phase 3: projection ----------------
    with tc.tile_pool(name="ob3", bufs=4) as ob3:
        for m in range(8):
            for tb in range(4):
                ps = scp.tile([128, 512], F32, tag="sc", name=f"p3_{m}_{tb}")
                for ct in range(4):
                    nc.tensor.matmul(
                        ps[:],
                        lhsT=wp_sb[:, ct, 128 * m:128 * (m + 1)],
                        rhs=ctxT_sb[:, ct, 512 * tb:512 * (tb + 1)],
                        start=(ct == 0), stop=(ct == 3))
                ob = ob3.tile([128, 512], F32, tag="o3", name=f"ob_{m}_{tb}")
                nc.vector.tensor_copy(ob[:], ps[:])
                nc.sync.dma_start(
                    out=outT.ap()[128 * m:128 * (m + 1), 512 * tb:512 * (tb + 1)],
                    in_=ob[:])

    _nrm_cm.__exit__(None, None, None)
    _prp_cm.__exit__(None, None, None)
    _cxp_cm.__exit__(None, None, None)
    _scp_cm.__exit__(None, None, None)
    _const_cm.__exit__(None, None, None)


_CACHED = {}


def _build(reps=1, layout="fill"):
    key = (reps, layout)
    if key in _CACHED:
        return _CACHED[key]
    nc = bacc.Bacc()
    xT = nc.dram_tensor("xT", [D, S], BF16, kind="ExternalInput")
    wq = nc.dram_tensor("wq", [D, GC], BF16, kind="ExternalInput")
    wk = nc.dram_tensor("wk", [D, GC], BF16, kind="ExternalInput")
    wv = nc.dram_tensor("wv", [D, GC], BF16, kind="ExternalInput")
    wp = nc.dram_tensor("wp", [GC, D], BF16, kind="ExternalInput")
    bqkv = nc.dram_tensor("bqkv", [1, 3 * GC], BF16, kind="ExternalInput")
    tri = nc.dram_tensor("tri", [128, 128], BF16, kind="ExternalInput")
    ident = nc.dram_tensor("ident", [128, 128], BF16, kind="ExternalInput")
    outT = nc.dram_tensor("outT", [D, S], F32, kind="ExternalOutput")
    with tile.TileContext(nc) as tc:
        for _ in range(reps):
            _body(nc, xT, wq, wk, wv, wp, bqkv, tri, ident, outT, tc, layout=layout)
    nc.compile()
    _CACHED[key] = nc
    return nc


def make_in_maps(x, W_attn, b_attn, W_proj):
    bf = ml_dtypes.bfloat16
    tri_np = np.where(np.arange(128)[None, :] >= np.arange(128)[:, None],
                      np.float32(0.0), np.float32(-1e9)).astype(bf)
    ident_np = np.eye(128, dtype=np.float32).astype(bf)
    in_maps = []
    for core in range(N_CORES):
        b, g = divmod(core, 2)
        cols = slice(GC * g, GC * (g + 1))
        in_maps.append({
            "xT": np.ascontiguousarray(x[b].T).astype(bf),
            "wq": np.ascontiguousarray(W_attn[:, cols]).astype(bf),
            "wk": np.ascontiguousarray(W_attn[:, D:][:, cols]).astype(bf),
            "wv": np.ascontiguousarray(W_attn[:, 2 * D:][:, cols]).astype(bf),
            "wp": np.ascontiguousarray(W_proj[cols, :]).astype(bf),
            "bqkv": np.concatenate(
                [b_attn[cols], b_attn[D:][cols], b_attn[2 * D:][cols]]
            ).reshape(1, 3 * GC).astype(bf),
            "tri": tri_np,
            "ident": ident_np,
        })
    return in_maps


def kernel(x, W_attn, b_attn, W_proj, b_proj, _run_kwargs=None):
    x = np.asarray(x)
    W_attn = np.asarray(W_attn)
    b_attn = np.asarray(b_attn)
    W_proj = np.asarray(W_proj)
    b_proj = np.asarray(b_proj)

    nc = _build()
    in_maps = make_in_maps(x, W_attn, b_attn, W_proj)

    res = run_bass_kernel_spmd(
        nc, in_maps, core_ids=list(range(N_CORES)), **(_run_kwargs or {}))

    out = np.empty((B, S, D), np.float32)
    for b in range(B):
        acc = res.results[2 * b]["outT"] + res.results[2 * b + 1]["outT"]
        out[b] = acc.T + b_proj[None, :].astype(np.float32)
    if _run_kwargs:
        kernel.last_results = res
    return out


# revision 21
# speedup vs baseline: 1.4477x; 1.4477x over previous
"""Causal self-attention (B=4, S=2048, D=1024, H=16) on 8 NeuronCores.

Sharding: core c handles batch b = c//2 and head-group g = c%2 (8 heads).
Each core computes qkv for its head group, causal attention for its 8 heads,
and a partial projection (its 512 rows of W_proj). Host sums the two partial
outputs per batch and adds b_proj.

Device layout notes:
 - x is passed transposed (xT [D, S]) and bf16; qT/kT are computed in
   [qkv_col, token] layout so the scores matmul needs no transposes:
   scoresT[k_tok, q_tok] = kT_tile.T @ qT  (lhsT = kT, contraction = head dim).
 - softmax runs on scoresT: exp on ScalarE (scale=1/8 folded in); the causal
   mask is applied in PSUM by adding a 0/-1e9 triangular tile to the diagonal
   128-wide band via an identity matmul (PE), and the masked prefix of each
   k-tile row is simply never computed or accumulated.
 - denominators come from a ones-column appended to v (v_aug [k,65]); the
   ctx matmul then yields [ctx(64 rows); sums(1 row)] per q block.
 - normalization: reciprocal of the sums row, partition-broadcast on GpSimd,
   one fused multiply+cast on VectorE.
"""

import numpy as np
import ml_dtypes

import concourse.bacc as bacc
import concourse.tile as tile
from concourse import mybir
from concourse.bass_utils import run_bass_kernel_spmd

BF16 = mybir.dt.bfloat16
F32 = mybir.dt.float32
EXP = mybir.ActivationFunctionType.Exp

B = 4
S = 2048  # tokens per batch
D = 1024
HG = 8    # heads per core
HD = 64
GC = HG * HD  # 512 qkv columns per core per q/k/v
N_CORES = 8
SCALE = 0.125  # 1/sqrt(64)


def _body(nc, xT, wq, wk, wv, wp, bqkv, tri, ident, outT, tc, layout="fill", use_bias=True):
    _const_cm = tc.tile_pool(name="const", bufs=1)
    const = _const_cm.__enter__()
    qT_sb = const.tile([128, 4, S], BF16)
    kT_sb = const.tile([128, 4, S], BF16)
    ctxT_sb = const.tile([128, 4, S], BF16)
    vaug_sb = const.tile([128, 16, HG, 65], BF16)
    wp_sb = const.tile([128, 4, D], BF16)
    tri_sb = const.tile([128, 128], BF16)
    ident_sb = const.tile([128, 128], BF16)
    b_sb = const.tile([1, 3 * GC], BF16)
    ones1 = const.tile([1, 512], BF16)

    nc.vector.memset(ones1[:], 1.0)
    nc.vector.memset(vaug_sb[:, :, :, 64:65], 1.0)
    nc.sync.dma_start(out=tri_sb[:], in_=tri.ap())
    nc.sync.dma_start(out=ident_sb[:], in_=ident.ap())
    nc.sync.dma_start(out=b_sb[:], in_=bqkv.ap())
    for ct in range(4):
        nc.sync.dma_start(out=wp_sb[:, ct, :], in_=wp.ap()[128 * ct:128 * (ct + 1), :])

    xT_sb = const.tile([128, 8, S], BF16)
    wq_sb = const.tile([128, 8, GC], BF16)
    wk_sb = const.tile([128, 8, GC], BF16)
    wv_sb = const.tile([128, 8, GC], BF16)
    # xT + wq stream first so the first qk matmuls can start ASAP
    for t in range(8):
        nc.sync.dma_start(out=xT_sb[:, t, :], in_=xT.ap()[128 * t:128 * (t + 1), :])
        nc.sync.dma_start(out=wq_sb[:, t, :], in_=wq.ap()[128 * t:128 * (t + 1), :])
    for t in range(8):
        nc.sync.dma_start(out=wk_sb[:, t, :], in_=wk.ap()[128 * t:128 * (t + 1), :])
    for t in range(8):
        nc.sync.dma_start(out=wv_sb[:, t, :], in_=wv.ap()[128 * t:128 * (t + 1), :])

    # One shared PSUM pool scheme across all phases so emission can pipeline:
    #   scp "sc": [128,1024] slots x2 (4 banks) - qkv psums, scores, proj
    #   cxp "cx": [65,512] slots x4 (4 banks)   - ctx accumulators
    _scp_cm = tc.tile_pool(name="scp", bufs=2, space="PSUM")
    scp = _scp_cm.__enter__()
    _cxp_cm = tc.tile_pool(name="cxp", bufs=4, space="PSUM")
    cxp = _cxp_cm.__enter__()
    _prp_cm = tc.tile_pool(name="prp", bufs=5)
    prp = _prp_cm.__enter__()
    _nrm_cm = tc.tile_pool(name="nrm", bufs=4)
    nrm = _nrm_cm.__enter__()

    def qk_group(c, qk, tb):
        w_sb, dst, boff = ((wq_sb, qT_sb, 0), (wk_sb, kT_sb, GC))[qk]
        ps = scp.tile([128, 512], F32, tag="sc", name=f"qk_{c}_{boff}_{tb}")
        for t in range(8):
            nc.tensor.matmul(
                ps[:],
                lhsT=w_sb[:, t, 128 * c:128 * (c + 1)],
                rhs=xT_sb[:, t, 512 * tb:512 * (tb + 1)],
                start=(t == 0), stop=(not use_bias and t == 7))
        if use_bias:
            nc.tensor.matmul(
                ps[:],
                lhsT=b_sb[0:1, boff + 128 * c: boff + 128 * (c + 1)],
                rhs=ones1[0:1, :],
                start=False, stop=True)
        nc.vector.tensor_copy(dst[:, c, 512 * tb:512 * (tb + 1)], ps[:])

    def v_tile(j):
        # v in natural [token, v_col] layout, + bias, scattered into v_aug
        psv = scp.tile([128, 512], F32, tag="sc", name=f"pv_{j}")
        for t in range(8):
            nc.tensor.matmul(
                psv[:],
                lhsT=xT_sb[:, t, 128 * j:128 * (j + 1)],
                rhs=wv_sb[:, t, :],
                start=(t == 0), stop=(not use_bias and t == 7))
        if use_bias:
            nc.tensor.matmul(
                psv[:],
                lhsT=ones1[0:1, 0:128],
                rhs=b_sb[0:1, 2 * GC:3 * GC],
                start=False, stop=True)
        nc.vector.tensor_copy(
            vaug_sb[:, j, :, 0:64],
            psv[:].rearrange("p (h c) -> p h c", h=HG))

    def normalize(h, qb, ctx_ps):
        o = 64 * (h % 2)
        c = h // 2
        rec = nrm.tile([1, 512], F32, tag="rec", name=f"rec_{h}_{qb}")
        nc.vector.reciprocal(rec[:], ctx_ps[qb][64:65, :])
        bc = nrm.tile([64, 512], F32, tag="bc", name=f"bc_{h}_{qb}")
        nc.gpsimd.partition_broadcast(bc[:], rec[:])
        if o == 0:
            nc.vector.tensor_mul(
                ctxT_sb[0:64, c, 512 * qb:512 * (qb + 1)],
                ctx_ps[qb][0:64, :], bc[:])
        else:
            stg = nrm.tile([64, 512], BF16, tag="stg", name=f"stg_{h}_{qb}")
            nc.vector.tensor_mul(stg[:], ctx_ps[qb][0:64, :], bc[:])
            nc.sync.dma_start(
                out=ctxT_sb[64:128, c, 512 * qb:512 * (qb + 1)], in_=stg[:])

    _ob3_cm = tc.tile_pool(name="ob3", bufs=4)
    ob3 = _ob3_cm.__enter__()

    def proj_group(m, tb):
        ps = scp.tile([128, 512], F32, tag="sc", name=f"p3_{m}_{tb}")
        for ct in range(4):
            nc.tensor.matmul(
                ps[:],
                lhsT=wp_sb[:, ct, 128 * m:128 * (m + 1)],
                rhs=ctxT_sb[:, ct, 512 * tb:512 * (tb + 1)],
                start=(ct == 0), stop=(ct == 3))
        ob = ob3.tile([128, 512], F32, tag="o3", name=f"ob_{m}_{tb}")
        nc.vector.tensor_copy(ob[:], ps[:])
        nc.sync.dma_start(
            out=outT.ap()[128 * m:128 * (m + 1), 512 * tb:512 * (tb + 1)],
            in_=ob[:])

    def h7_filler(j):
        # tb-block tb of the projection becomes legal once head 7's q-block
        # tb is normalized at j = 4*tb + 3; emit 2 (m, tb) groups per j
        if j >= 4:
            idx = j - 4
            tb, pair = idx // 4, idx % 4
            proj_group(2 * pair, tb)
            proj_group(2 * pair + 1, tb)

    def head_block(h, filler=None):
        o = 64 * (h % 2)
        c = h // 2
        ctx_ps = [cxp.tile([65, 512], F32, tag="cx", name=f"cx_{h}_{qb}")
                  for qb in range(4)]
        for j in range(16):
            if filler is not None:
                filler(j)
            qbm, r = divmod(j, 4)
            width = S - 512 * qbm
            rel0 = 128 * r
            pT = prp.tile([128, S], BF16, tag="probs", name=f"pT_{h}_{j}")
            # scores chunks of <=1024 free, one exp per chunk; the causal mask
            # is applied in PSUM by adding tri_neg (0 / -1e9) to the diagonal
            # 128-wide band via an identity matmul, keeping the whole
            # scores->exp chain on PE->ACT only
            for ch0 in range(0, width, 1024):
                ch1 = min(ch0 + 1024, width)
                lo = max(ch0, rel0)
                if lo >= ch1:
                    continue
                ps = scp.tile([128, 1024], F32, tag="sc", name=f"sc_{h}_{j}_{ch0}")
                for qb in range(qbm + ch0 // 512, qbm + ch1 // 512):
                    rq0 = (qb - qbm) * 512
                    mlo = max(rq0, rel0)
                    diag = mlo == rel0 and ch0 == 0
                    nc.tensor.matmul(
                        ps[:, mlo - ch0: rq0 + 512 - ch0],
                        lhsT=kT_sb[o:o + 64, c, 128 * j:128 * (j + 1)],
                        rhs=qT_sb[o:o + 64, c,
                                  512 * qbm + mlo: 512 * qbm + rq0 + 512],
                        start=True, stop=not diag, skip_group_check=True)
                    if diag:
                        nc.tensor.matmul(
                            ps[:, rel0 - ch0: rel0 - ch0 + 128],
                            lhsT=ident_sb[:],
                            rhs=tri_sb[:],
                            start=False, stop=True, skip_group_check=True)
                nc.scalar.activation(
                    pT[:, lo:ch1], ps[:, lo - ch0:ch1 - ch0], EXP, scale=SCALE)
            # ctx accumulation (with sums in row 64); the diagonal block's
            # masked prefix [0, rel0) is never computed nor accumulated
            for qb in range(qbm, 4):
                lo = rel0 if qb == qbm else 0
                nc.tensor.matmul(
                    ctx_ps[qb][:, lo:512],
                    lhsT=vaug_sb[:, j, h, :],
                    rhs=pT[:, (qb - qbm) * 512 + lo: (qb - qbm + 1) * 512],
                    start=(j == 0), stop=(j == 4 * qb + 3))
            if r == 3:
                # qb = (j-3)//4 just received its last accumulation
                normalize(h, (j - 3) // 4, ctx_ps)

    def spread(groups):
        stride = max(1, 16 // max(1, len(groups)))
        def f(j):
            i = j // stride
            if j % stride == 0 and i < len(groups):
                groups[i]()
        return f

    qkg = [[(lambda c=c, qk=qk, tb=tb: qk_group(c, qk, tb))
            for qk in range(2) for tb in range(4)] for c in range(4)]
    if layout == "fill":
        # qk(0) upfront; v interleaved into h0 two iterations ahead of use;
        # qk(1..3) spread into h1..h5
        for g in qkg[0]:
            g()
        v_tile(0)
        v_tile(1)
        head_block(0, filler=lambda j: v_tile(j + 2) if j < 14 else None)
        head_block(1, filler=spread(qkg[1]))
        head_block(2, filler=spread(qkg[2][:4]))
        head_block(3, filler=spread(qkg[2][4:]))
        head_block(4, filler=spread(qkg[3][:4]))
        head_block(5, filler=spread(qkg[3][4:]))
        head_block(6)
        head_block(7, filler=h7_filler)
    elif layout == "seq":
        # all qkv upfront, then pure attention heads
        for c in range(4):
            for g in qkg[c]:
                g()
        for j in range(16):
            v_tile(j)
        for h in range(HG - 1):
            head_block(h)
        head_block(7, filler=h7_filler)
    elif layout == "block":
        # qkv blocks between head pairs
        for g in qkg[0]:
            g()
        for j in range(16):
            v_tile(j)
        for c in range(4):
            if c:
                for g in qkg[c]:
                    g()
            head_block(2 * c)
            head_block(2 * c + 1, filler=h7_filler if c == 3 else None)
    else:
        raise ValueError(layout)


    for pair in range(4):
        proj_group(2 * pair, 3)
        proj_group(2 * pair + 1, 3)

    _ob3_cm.__exit__(None, None, None)
    _nrm_cm.__exit__(None, None, None)
    _prp_cm.__exit__(None, None, None)
    _cxp_cm.__exit__(None, None, None)
    _scp_cm.__exit__(None, None, None)
    _const_cm.__exit__(None, None, None)


_CACHED = {}


def _build(reps=1, layout="fill", use_bias=True):
    key = (reps, layout, use_bias)
    if key in _CACHED:
        return _CACHED[key]
    nc = bacc.Bacc()
    xT = nc.dram_tensor("xT", [D, S], BF16, kind="ExternalInput")
    wq = nc.dram_tensor("wq", [D, GC], BF16, kind="ExternalInput")
    wk = nc.dram_tensor("wk", [D, GC], BF16, kind="ExternalInput")
    wv = nc.dram_tensor("wv", [D, GC], BF16, kind="ExternalInput")
    wp = nc.dram_tensor("wp", [GC, D], BF16, kind="ExternalInput")
    bqkv = nc.dram_tensor("bqkv", [1, 3 * GC], BF16, kind="ExternalInput")
    tri = nc.dram_tensor("tri", [128, 128], BF16, kind="ExternalInput")
    ident = nc.dram_tensor("ident", [128, 128], BF16, kind="ExternalInput")
    outT = nc.dram_tensor("outT", [D, S], F32, kind="ExternalOutput")
    with tile.TileContext(nc) as tc:
        for _ in range(reps):
            _body(nc, xT, wq, wk, wv, wp, bqkv, tri, ident, outT, tc, layout=layout, use_bias=use_bias)
    nc.compile()
    _CACHED[key] = nc
    return nc


def make_in_maps(x, W_attn, b_attn, W_proj):
    bf = ml_dtypes.bfloat16
    tri_np = np.where(np.arange(128)[None, :] >= np.arange(128)[:, None],
                      np.float32(0.0), np.float32(-1e9)).astype(bf)
    ident_np = np.eye(128, dtype=np.float32).astype(bf)
    in_maps = []
    for core in range(N_CORES):
        b, g = divmod(core, 2)
        cols = slice(GC * g, GC * (g + 1))
        in_maps.append({
            "xT": np.ascontiguousarray(x[b].T).astype(bf),
            "wq": np.ascontiguousarray(W_attn[:, cols]).astype(bf),
            "wk": np.ascontiguousarray(W_attn[:, D:][:, cols]).astype(bf),
            "wv": np.ascontiguousarray(W_attn[:, 2 * D:][:, cols]).astype(bf),
            "wp": np.ascontiguousarray(W_proj[cols, :]).astype(bf),
            "bqkv": np.concatenate(
                [b_attn[cols], b_attn[D:][cols], b_attn[2 * D:][cols]]
            ).reshape(1, 3 * GC).astype(bf),
            "tri": tri_np,
            "ident": ident_np,
        })
    return in_maps


def kernel(x, W_attn, b_attn, W_proj, b_proj, _run_kwargs=None):
    x = np.asarray(x)
    W_attn = np.asarray(W_attn)
    b_attn = np.asarray(b_attn)
    W_proj = np.asarray(W_proj)
    b_proj = np.asarray(b_proj)

    use_bias = bool(np.any(b_attn))
    nc = _build(use_bias=use_bias)
    in_maps = make_in_maps(x, W_attn, b_attn, W_proj)

    res = run_bass_kernel_spmd(
        nc, in_maps, core_ids=list(range(N_CORES)), **(_run_kwargs or {}))

    out = np.empty((B, S, D), np.float32)
    for b in range(B):
        acc = res.results[2 * b]["outT"] + res.results[2 * b + 1]["outT"]
        out[b] = acc.T + b_proj[None, :].astype(np.float32)
    if _run_kwargs:
        kernel.last_results = res
    return out
